# revision 22
# baseline (speedup 1.0000x reference)
"""Trainium2 Bass kernel for nn_Entropy (KDE local-entropy via histogram binning).

Contract: kernel(**inputs) takes the FULL input x (2,2,1,80,80) fp32 and
returns the FULL output (2,2,80,80) fp32, sharding internally across 8
NeuronCores (core = batch*2 + row-half of the 74x74 patch grid).

Algorithm (per core, one 47x80 input strip -> 37x74 entropy block):
  1. unsharp preprocessing (5x5 blur via PE banded matmul + free-axis tree
     adds, exact-tie-aware rounding, IEEE-reciprocal division) -> integer
     "division" image in [0,255].
  2. KDE entropy: per 7x7 patch the pairwise-exp sum collapses onto a
     256-bin histogram h; G = K @ h with the constant 256x256 kernel matrix
     K[b,b'] = exp(-(b-b')^2/12.5); ent = -(1/49) * h . log(G/(49*norm)+eps).
     h is built from a one-hot image (bins on partitions) box-summed with
     prefix-scan + shifted-subtract in both spatial directions.
"""
import os
import sys

import numpy as np

for _p in ("/opt/trn_rl_repo", "/root/.axon_site/_ro/trn_rl_repo"):
    if os.path.isdir(_p) and _p not in sys.path:
        sys.path.insert(0, _p)

import concourse.bass as bass
import concourse.bacc as bacc
import concourse.tile as tile
from concourse import mybir
from concourse.bass_utils import run_bass_kernel_spmd

dt = mybir.dt
Alu = mybir.AluOpType
Act = mybir.ActivationFunctionType
f32 = np.float32

R = 7
BW = 2.5
L = R * R  # 49
NORM = f32((2.0 * np.pi * BW * BW) ** 0.5)  # C=1 -> exponent 1/2
LN_SCALE = float(f32(1.0 / (L * NORM)))
INV25 = float(f32(1.0) / f32(25.0))
NEG_INV_L = float(-(f32(1.0) / f32(L)))

# geometry
HP = 74          # patch grid cols (80 - 7 + 1)
ROWS = 43        # division-image rows needed per core (37 patch rows + 6)
PR = 37          # patch rows per core
NPIX = ROWS * 80         # 3440
NHR = ROWS * HP          # 3182 horizontal-sum elements
NP_ = PR * HP            # 2738 patches per core
CHUNK = 512

_COMPILED = None  # (nc, const_inputs)


def _host_constants():
    f16 = np.float16
    bins = np.arange(256, dtype=np.float64)
    kmat = np.exp(-((bins[:, None] - bins[None, :]) ** 2) / (2.0 * BW * BW)).astype(f16)
    b5 = np.zeros((47, ROWS), f32)
    for m in range(ROWS):
        b5[m : m + 5, m] = 1.0
    binsA = np.arange(0, 128, dtype=f32).reshape(128, 1)
    binsB = np.arange(128, 256, dtype=f32).reshape(128, 1)
    ones = np.ones((128, 1), f16)
    onesrow = np.ones((1, 128), f16)
    return {
        "kmat": kmat, "b5": b5, "binsA": binsA, "binsB": binsB,
        "ones": ones, "onesrow": onesrow,
    }


def _build_nc():
    nc = bacc.Bacc("TRN2", target_bir_lowering=False, debug=False)

    xs_d = nc.dram_tensor("xs", [47, 80], dt.float32, kind="ExternalInput")
    xm_d = nc.dram_tensor("xm", [ROWS, 80], dt.float32, kind="ExternalInput")
    b5_d = nc.dram_tensor("b5", [47, ROWS], dt.float32, kind="ExternalInput")
    kmat_d = nc.dram_tensor("kmat", [256, 256], dt.float16, kind="ExternalInput")
    binsA_d = nc.dram_tensor("binsA", [128, 1], dt.float32, kind="ExternalInput")
    binsB_d = nc.dram_tensor("binsB", [128, 1], dt.float32, kind="ExternalInput")
    ones_d = nc.dram_tensor("ones", [128, 1], dt.float16, kind="ExternalInput")
    onesrow_d = nc.dram_tensor("onesrow", [1, 128], dt.float16, kind="ExternalInput")
    ent_d = nc.dram_tensor("ent", [NP_], dt.float32, kind="ExternalOutput")

    chunks = []
    off = 0
    while off < NP_:
        cw = min(CHUNK, NP_ - off)
        chunks.append((off, cw))
        off += cw

    with tile.TileContext(nc) as tc:
        with (
            tc.tile_pool(name="small", bufs=1) as small,
            tc.tile_pool(name="pre", bufs=1) as pre,
            tc.tile_pool(name="big", bufs=1) as big,
            tc.tile_pool(name="scratch", bufs=1) as scratch,
            tc.tile_pool(name="psum", bufs=4, space="PSUM") as psum,
            tc.tile_pool(name="psum1", bufs=2, space="PSUM") as psum1,
        ):
            # ---------- stage A: preprocessing -> division [43, 80] ----------
            # NB: every matmul operand below is last-written by the vector
            # engine (copies after DMA) so LDWEIGHTS carries <=2 sync waits
            # (walrus limit).
            xt = pre.tile([47, 84], dt.float32)
            nc.vector.memset(xt[:], 0.0)
            nc.sync.dma_start(xt[:, 2:82], xs_d[:])
            xmid = pre.tile([ROWS, 80], dt.float32)
            nc.sync.dma_start(xmid[:], xm_d[:])
            b5t = pre.tile([47, ROWS], dt.float32)
            nc.sync.dma_start(b5t[:], b5_d[:])

            sv_ps = psum1.tile([ROWS, 84], dt.float32, tag="mps")
            nc.tensor.matmul(sv_ps[:], b5t[:], xt[:], start=True, stop=True)
            sv = pre.tile([ROWS, 84], dt.float32)
            nc.scalar.copy(sv[:], sv_ps[:])

            # horizontal 5-sum tree: t1=2sum, t2=4sum, s25=t2+shift4
            t1 = pre.tile([ROWS, 83], dt.float32)
            nc.vector.tensor_add(t1[:], sv[:, 0:83], sv[:, 1:84])
            t2 = pre.tile([ROWS, 81], dt.float32)
            nc.vector.tensor_add(t2[:], t1[:, 0:81], t1[:, 2:83])
            s25 = pre.tile([ROWS, 80], dt.float32)
            nc.vector.tensor_add(s25[:], t2[:, 0:80], sv[:, 4:84])

            # RNE(v) == (v + 2^23) - 2^23 for 0 <= v < 2^23 (bit-exact
            # round-half-even, matching jnp.round). Keep the two magic ops in
            # separate instructions so the intermediate rounds to fp32.
            MAGIC = 8388608.0

            # smooth = RNE(s25/25)
            tt = pre.tile([ROWS, 80], dt.float32)
            nc.vector.tensor_scalar(tt[:], s25[:], INV25, MAGIC, Alu.mult, Alu.add)
            smooth = pre.tile([ROWS, 80], dt.float32)
            nc.vector.tensor_scalar(smooth[:], tt[:], MAGIC, None, Alu.subtract)

            # sharp = RNE(clip(2.5 x - 1.25 smooth, 0, 255))
            aa = pre.tile([ROWS, 80], dt.float32)
            nc.vector.tensor_scalar(aa[:], xmid[:], 2.5, None, Alu.mult)
            sp = pre.tile([ROWS, 80], dt.float32)
            nc.vector.scalar_tensor_tensor(
                sp[:], smooth[:], -1.25, aa[:], Alu.mult, Alu.add
            )
            nc.vector.tensor_scalar(sp[:], sp[:], 0.0, 255.0, Alu.max, Alu.min)
            nc.vector.tensor_scalar(tt[:], sp[:], MAGIC, None, Alu.add)
            sharp = pre.tile([ROWS, 80], dt.float32)
            nc.vector.tensor_scalar(sharp[:], tt[:], MAGIC, None, Alu.subtract)

            # division = min(RNE(sharp*255 * recip(smooth+1e-8)), 255)
            denom = pre.tile([ROWS, 80], dt.float32)
            nc.vector.tensor_scalar(denom[:], smooth[:], 1e-8, None, Alu.add)
            rr = pre.tile([ROWS, 80], dt.float32)
            nc.vector.reciprocal(rr[:], denom[:])
            vv = pre.tile([ROWS, 80], dt.float32)
            nc.vector.scalar_tensor_tensor(
                vv[:], sharp[:], 255.0, rr[:], Alu.mult, Alu.mult
            )
            nc.vector.tensor_scalar(tt[:], vv[:], MAGIC, None, Alu.add)
            dv = pre.tile([ROWS, 80], dt.float16)
            nc.vector.tensor_scalar(dv[:], tt[:], MAGIC, 255.0, Alu.subtract, Alu.min)

            # ---------- stages B+C, split into two column blocks so the
            # DVE box-sum trees of one block overlap the PE/ACT broadcast and
            # G/Ln/dot work of the other ----------------------------------
            onesrow_t = small.tile([1, 128], dt.float16)
            nc.sync.dma_start(onesrow_t[:], onesrow_d[:])
            binsA_t = small.tile([128, 1], dt.float32)
            nc.sync.dma_start(binsA_t[:], binsA_d[:])
            binsB_t = small.tile([128, 1], dt.float32)
            nc.sync.dma_start(binsB_t[:], binsB_d[:])
            ones_t = small.tile([128, 1], dt.float16)
            nc.sync.dma_start(ones_t[:], ones_d[:])
            eps_t = small.tile([128, 1], dt.float32)
            nc.vector.memset(eps_t[:], 1e-8)

            kt = {}
            for bi in range(2):
                for bo in range(2):
                    kt[bi, bo] = small.tile(
                        [128, 128], dt.float16, tag=f"k{bi}{bo}", name=f"k{bi}{bo}"
                    )
                    nc.sync.dma_start(
                        kt[bi, bo][:],
                        kmat_d[bi * 128 : (bi + 1) * 128, bo * 128 : (bo + 1) * 128],
                    )


            # (img_col_lo, img_col_hi, patch_col_lo): each block covers 43
            # image cols -> 37 patch cols
            for blk, (c0, c1, pc0) in enumerate([(0, 43, 0), (37, 80, 37)]):
                cb = c1 - c0          # 43 image cols
                pcb = cb - R + 1      # 37 patch cols
                npx = ROWS * cb       # 1849 pixels
                npb = PR * pcb        # 1369 patches

                ent_blk = small.tile(
                    [1, npb], dt.float32, tag="ent_blk", name="ent_blk", bufs=2
                )
                dvrow = small.tile(
                    [1, npx], dt.float16, tag="dvrow", name="dvrow", bufs=2
                )
                nc.sync.dma_start(dvrow[:], dv[:, c0:c1])

                # broadcast + fused one-hot oh[p, hb, r, cc] chunk-by-chunk
                dvbc = scratch.tile([128, npx], dt.float16, tag="dvbc", name="dvbc")
                oh = scratch.tile([128, 2 * npx], dt.float16, tag="oh", name="oh")
                boff = 0
                while boff < npx:
                    bw = min(CHUNK, npx - boff)
                    bc_ps = psum.tile([128, bw], dt.float32, tag="g_ps", name="bc_ps")
                    nc.tensor.matmul(
                        bc_ps[:], onesrow_t[:], dvrow[:, boff : boff + bw],
                        start=True, stop=True,
                    )
                    nc.scalar.copy(dvbc[:, boff : boff + bw], bc_ps[:])
                    nc.vector.tensor_scalar(
                        oh[:, boff : boff + bw],
                        dvbc[:, boff : boff + bw], binsA_t[:], None, Alu.is_equal,
                    )
                    nc.vector.tensor_scalar(
                        oh[:, npx + boff : npx + boff + bw],
                        dvbc[:, boff : boff + bw], binsB_t[:], None, Alu.is_equal,
                    )
                    boff += bw
                oh4 = oh[:].rearrange("p (h r c) -> p h r c", h=2, r=ROWS, c=cb)

                # vertical 7-sum tree over r (43 -> 37): 7 = 4 + 2 + 1
                v1 = scratch.tile([128, 2 * 42 * cb], dt.float16, tag="v1", name="v1")
                v1v = v1[:].rearrange("p (h r c) -> p h r c", h=2, r=42, c=cb)
                nc.vector.tensor_add(v1v, oh4[:, :, 0:42, :], oh4[:, :, 1:43, :])
                v2 = scratch.tile([128, 2 * 40 * cb], dt.float16, tag="v2", name="v2")
                v2v = v2[:].rearrange("p (h r c) -> p h r c", h=2, r=40, c=cb)
                nc.vector.tensor_add(v2v, v1v[:, :, 0:40, :], v1v[:, :, 2:42, :])
                u2 = scratch.tile([128, 2 * PR * cb], dt.float16, tag="u2", name="u2")
                u2v = u2[:].rearrange("p (h r c) -> p h r c", h=2, r=PR, c=cb)
                nc.vector.tensor_add(
                    u2v, v2v[:, :, 0:PR, :], v1v[:, :, 4 : 4 + PR, :]
                )
                v7 = scratch.tile([128, 2 * PR * cb], dt.float16, tag="v7", name="v7")
                v7v = v7[:].rearrange("p (h r c) -> p h r c", h=2, r=PR, c=cb)
                nc.vector.tensor_add(v7v, u2v, oh4[:, :, 6:43, :])

                # horizontal 7-sum tree over cc (43 -> 37)
                t1h = scratch.tile(
                    [128, 2 * PR * (cb - 1)], dt.float16, tag="t1h", name="t1h"
                )
                t1v = t1h[:].rearrange("p (h r c) -> p h r c", h=2, r=PR, c=cb - 1)
                nc.vector.tensor_add(t1v, v7v[:, :, :, 0 : cb - 1], v7v[:, :, :, 1:cb])
                t2h = scratch.tile(
                    [128, 2 * PR * (cb - 3)], dt.float16, tag="t2h", name="t2h"
                )
                t2v = t2h[:].rearrange("p (h r c) -> p h r c", h=2, r=PR, c=cb - 3)
                nc.vector.tensor_add(
                    t2v, t1v[:, :, :, 0 : cb - 3], t1v[:, :, :, 2 : cb - 1]
                )
                uh = scratch.tile(
                    [128, 2 * PR * pcb], dt.float16, tag="uh", name="uh"
                )
                uhv = uh[:].rearrange("p (h r c) -> p h r c", h=2, r=PR, c=pcb)
                nc.vector.tensor_add(
                    uhv, t2v[:, :, :, 0:pcb], t1v[:, :, :, 4 : 4 + pcb]
                )
                h_f = big.tile(
                    [128, 2 * npb], dt.float16, tag="h_f", name="h_f", bufs=2
                )
                hfv = h_f[:].rearrange("p (h r c) -> p h r c", h=2, r=PR, c=pcb)
                nc.vector.tensor_add(hfv, uhv, v7v[:, :, :, 6:cb])

                # stage C for this block, chunked by patch-row groups
                RG = 13  # 13*37 = 481 <= 512 PSUM bank limit
                r0 = 0
                while r0 < PR:
                    r1 = min(r0 + RG, PR)
                    off = r0 * pcb
                    cw = (r1 - r0) * pcb
                    h0c = h_f[:, off : off + cw]
                    h1c = h_f[:, npb + off : npb + off + cw]
                    g0 = psum.tile([128, cw], dt.float32, tag="g_ps", name="g0")
                    nc.tensor.matmul(g0[:], kt[0, 0][:], h0c, start=True, stop=False)
                    nc.tensor.matmul(g0[:], kt[1, 0][:], h1c, start=False, stop=True)
                    g1 = psum.tile([128, cw], dt.float32, tag="g_ps", name="g1")
                    nc.tensor.matmul(g1[:], kt[0, 1][:], h0c, start=True, stop=False)
                    nc.tensor.matmul(g1[:], kt[1, 1][:], h1c, start=False, stop=True)
                    lp0 = scratch.tile(
                        [128, cw], dt.float16, tag="lp0", name="lp0", bufs=3
                    )
                    nc.scalar.activation(
                        lp0[:], g0[:], Act.Ln, bias=eps_t[:], scale=LN_SCALE
                    )
                    lp1 = scratch.tile(
                        [128, cw], dt.float16, tag="lp1", name="lp1", bufs=3
                    )
                    nc.scalar.activation(
                        lp1[:], g1[:], Act.Ln, bias=eps_t[:], scale=LN_SCALE
                    )
                    m0 = scratch.tile(
                        [128, cw], dt.float16, tag="m0", name="m0", bufs=3
                    )
                    nc.vector.tensor_mul(m0[:], h0c, lp0[:])
                    m1 = scratch.tile(
                        [128, cw], dt.float16, tag="m1", name="m1", bufs=3
                    )
                    nc.vector.tensor_mul(m1[:], h1c, lp1[:])
                    psc = scratch.tile(
                        [128, cw], dt.float16, tag="psc", name="psc", bufs=3
                    )
                    nc.vector.tensor_add(psc[:], m0[:], m1[:])
                    e_ps = psum1.tile([1, cw], dt.float32, tag="mps")
                    nc.tensor.matmul(
                        e_ps[:], ones_t[:], psc[:], start=True, stop=True
                    )
                    nc.scalar.mul(ent_blk[:, off : off + cw], e_ps[:], NEG_INV_L)
                    r0 = r1
                nc.sync.dma_start(ent_d[blk * npb : (blk + 1) * npb], ent_blk[:])


    nc.compile()
    return nc


def _get_compiled():
    global _COMPILED
    if _COMPILED is None:
        _COMPILED = (_build_nc(), _host_constants())
    return _COMPILED


def _run(x, trace=False, **kw):
    """x: (2,2,1,80,80) float32. Returns (results, BassKernelResults)."""
    nc, consts = _get_compiled()
    xi = np.ascontiguousarray(np.asarray(x, f32).reshape(4, 80, 80))
    in_maps = []
    for core in range(8):
        b, half = core // 2, core % 2
        r0 = half * PR
        strip = np.zeros((47, 80), f32)
        lo, hi = r0 - 2, r0 + 45
        slo, shi = max(lo, 0), min(hi, 80)
        strip[slo - lo : shi - lo] = xi[b, slo:shi]
        m = dict(consts)
        m["xs"] = strip
        m["xm"] = np.ascontiguousarray(strip[2 : 2 + ROWS])
        in_maps.append(m)
    res = run_bass_kernel_spmd(nc, in_maps, list(range(8)), trace=trace, **kw)
    return res


def kernel(x):
    res = _run(x)
    out = np.zeros((4, 80, 80), f32)
    pad = R // 2
    for core in range(8):
        b, half = core // 2, core % 2
        r0 = half * PR
        er = np.asarray(res.results[core]["ent"], f32)
        ent = np.empty((PR, HP), f32)
        ent[:, 0:37] = er[0:1369].reshape(PR, 37)
        ent[:, 37:74] = er[1369:2738].reshape(PR, 37)
        out[b, pad + r0 : pad + r0 + PR, pad : pad + HP] = ent
    return out.reshape(2, 2, 80, 80)


# revision 24
# speedup vs baseline: 1.0947x; 1.0947x over previous
"""Trainium2 Bass kernel for nn_Entropy (KDE local-entropy via histogram binning).

Contract: kernel(**inputs) takes the FULL input x (2,2,1,80,80) fp32 and
returns the FULL output (2,2,80,80) fp32, sharding internally across 8
NeuronCores (core = batch*2 + row-half of the 74x74 patch grid).

Algorithm (per core, one 47x80 input strip -> 37x74 entropy block):
  1. unsharp preprocessing (5x5 blur via PE banded matmul + free-axis tree
     adds, exact-tie-aware rounding, IEEE-reciprocal division) -> integer
     "division" image in [0,255].
  2. KDE entropy: per 7x7 patch the pairwise-exp sum collapses onto a
     256-bin histogram h; G = K @ h with the constant 256x256 kernel matrix
     K[b,b'] = exp(-(b-b')^2/12.5); ent = -(1/49) * h . log(G/(49*norm)+eps).
     h is built from a one-hot image (bins on partitions) box-summed with
     prefix-scan + shifted-subtract in both spatial directions.
"""
import os
import sys

import numpy as np

for _p in ("/opt/trn_rl_repo", "/root/.axon_site/_ro/trn_rl_repo"):
    if os.path.isdir(_p) and _p not in sys.path:
        sys.path.insert(0, _p)

import concourse.bass as bass
import concourse.bacc as bacc
import concourse.tile as tile
from concourse import mybir
from concourse.bass_utils import run_bass_kernel_spmd

dt = mybir.dt
Alu = mybir.AluOpType
Act = mybir.ActivationFunctionType
f32 = np.float32

R = 7
BW = 2.5
L = R * R  # 49
NORM = f32((2.0 * np.pi * BW * BW) ** 0.5)  # C=1 -> exponent 1/2
LN_SCALE = float(f32(1.0 / (L * NORM)))
INV25 = float(f32(1.0) / f32(25.0))
NEG_INV_L = float(-(f32(1.0) / f32(L)))

# geometry
HP = 74          # patch grid cols (80 - 7 + 1)
ROWS = 43        # division-image rows needed per core (37 patch rows + 6)
PR = 37          # patch rows per core
NPIX = ROWS * 80         # 3440
NHR = ROWS * HP          # 3182 horizontal-sum elements
NP_ = PR * HP            # 2738 patches per core
CHUNK = 512

_COMPILED = None  # (nc, const_inputs)


def _host_constants():
    f16 = np.float16
    bins = np.arange(256, dtype=np.float64)
    kmat = np.exp(-((bins[:, None] - bins[None, :]) ** 2) / (2.0 * BW * BW)).astype(f16)
    b5 = np.zeros((47, ROWS), f32)
    for m in range(ROWS):
        b5[m : m + 5, m] = 1.0
    binsA = np.arange(0, 128, dtype=f32).reshape(128, 1)
    binsB = np.arange(128, 256, dtype=f32).reshape(128, 1)
    ones = np.ones((128, 1), f16)
    onesrow = np.ones((1, 128), f16)
    return {
        "kmat": kmat, "b5": b5, "binsA": binsA, "binsB": binsB,
        "ones": ones, "onesrow": onesrow,
    }


def _build_nc():
    nc = bacc.Bacc("TRN2", target_bir_lowering=False, debug=False)

    xs_d = nc.dram_tensor("xs", [47, 80], dt.float32, kind="ExternalInput")
    xm_d = nc.dram_tensor("xm", [ROWS, 80], dt.float32, kind="ExternalInput")
    b5_d = nc.dram_tensor("b5", [47, ROWS], dt.float32, kind="ExternalInput")
    kmat_d = nc.dram_tensor("kmat", [256, 256], dt.float16, kind="ExternalInput")
    binsA_d = nc.dram_tensor("binsA", [128, 1], dt.float32, kind="ExternalInput")
    binsB_d = nc.dram_tensor("binsB", [128, 1], dt.float32, kind="ExternalInput")
    ones_d = nc.dram_tensor("ones", [128, 1], dt.float16, kind="ExternalInput")
    onesrow_d = nc.dram_tensor("onesrow", [1, 128], dt.float16, kind="ExternalInput")
    ent_d = nc.dram_tensor("ent", [NP_], dt.float32, kind="ExternalOutput")

    chunks = []
    off = 0
    while off < NP_:
        cw = min(CHUNK, NP_ - off)
        chunks.append((off, cw))
        off += cw

    with tile.TileContext(nc) as tc:
        with (
            tc.tile_pool(name="small", bufs=1) as small,
            tc.tile_pool(name="pre", bufs=1) as pre,
            tc.tile_pool(name="big", bufs=1) as big,
            tc.tile_pool(name="scratch", bufs=1) as scratch,
            tc.tile_pool(name="psum", bufs=4, space="PSUM") as psum,
            tc.tile_pool(name="psum1", bufs=2, space="PSUM") as psum1,
        ):
            # ---------- stage A: preprocessing -> division [43, 80] ----------
            # NB: every matmul operand below is last-written by the vector
            # engine (copies after DMA) so LDWEIGHTS carries <=2 sync waits
            # (walrus limit).
            xt = pre.tile([47, 84], dt.float32)
            nc.vector.memset(xt[:], 0.0)
            nc.sync.dma_start(xt[:, 2:82], xs_d[:])
            xmid = pre.tile([ROWS, 80], dt.float32)
            nc.sync.dma_start(xmid[:], xm_d[:])
            b5t = pre.tile([47, ROWS], dt.float32)
            nc.sync.dma_start(b5t[:], b5_d[:])

            sv_ps = psum1.tile([ROWS, 84], dt.float32, tag="mps")
            nc.tensor.matmul(sv_ps[:], b5t[:], xt[:], start=True, stop=True)
            sv = pre.tile([ROWS, 84], dt.float32)
            nc.scalar.copy(sv[:], sv_ps[:])

            # horizontal 5-sum tree: t1=2sum, t2=4sum, s25=t2+shift4
            t1 = pre.tile([ROWS, 83], dt.float32)
            nc.vector.tensor_add(t1[:], sv[:, 0:83], sv[:, 1:84])
            t2 = pre.tile([ROWS, 81], dt.float32)
            nc.vector.tensor_add(t2[:], t1[:, 0:81], t1[:, 2:83])
            s25 = pre.tile([ROWS, 80], dt.float32)
            nc.vector.tensor_add(s25[:], t2[:, 0:80], sv[:, 4:84])

            # RNE(v) == (v + 2^23) - 2^23 for 0 <= v < 2^23 (bit-exact
            # round-half-even, matching jnp.round). Keep the two magic ops in
            # separate instructions so the intermediate rounds to fp32.
            MAGIC = 8388608.0

            # smooth = RNE(s25/25)
            tt = pre.tile([ROWS, 80], dt.float32)
            nc.vector.tensor_scalar(tt[:], s25[:], INV25, MAGIC, Alu.mult, Alu.add)
            smooth = pre.tile([ROWS, 80], dt.float32)
            nc.vector.tensor_scalar(smooth[:], tt[:], MAGIC, None, Alu.subtract)

            # sharp = RNE(clip(2.5 x - 1.25 smooth, 0, 255))
            aa = pre.tile([ROWS, 80], dt.float32)
            nc.vector.tensor_scalar(aa[:], xmid[:], 2.5, None, Alu.mult)
            sp = pre.tile([ROWS, 80], dt.float32)
            nc.vector.scalar_tensor_tensor(
                sp[:], smooth[:], -1.25, aa[:], Alu.mult, Alu.add
            )
            nc.vector.tensor_scalar(sp[:], sp[:], 0.0, 255.0, Alu.max, Alu.min)
            nc.vector.tensor_scalar(tt[:], sp[:], MAGIC, None, Alu.add)
            sharp = pre.tile([ROWS, 80], dt.float32)
            nc.vector.tensor_scalar(sharp[:], tt[:], MAGIC, None, Alu.subtract)

            # division = min(RNE(sharp*255 * recip(smooth+1e-8)), 255)
            denom = pre.tile([ROWS, 80], dt.float32)
            nc.vector.tensor_scalar(denom[:], smooth[:], 1e-8, None, Alu.add)
            rr = pre.tile([ROWS, 80], dt.float32)
            nc.vector.reciprocal(rr[:], denom[:])
            vv = pre.tile([ROWS, 80], dt.float32)
            nc.vector.scalar_tensor_tensor(
                vv[:], sharp[:], 255.0, rr[:], Alu.mult, Alu.mult
            )
            nc.vector.tensor_scalar(tt[:], vv[:], MAGIC, None, Alu.add)
            dv = pre.tile([ROWS, 80], dt.float16)
            nc.vector.tensor_scalar(dv[:], tt[:], MAGIC, 255.0, Alu.subtract, Alu.min)

            # ---------- stage B: broadcast + fused one-hot + 7x7 box trees ----
            dvrow = small.tile([1, NPIX], dt.float16)
            nc.sync.dma_start(dvrow[:], dv[:])
            onesrow_t = small.tile([1, 128], dt.float16)
            nc.sync.dma_start(onesrow_t[:], onesrow_d[:])
            binsA_t = small.tile([128, 1], dt.float32)
            nc.sync.dma_start(binsA_t[:], binsA_d[:])
            binsB_t = small.tile([128, 1], dt.float32)
            nc.sync.dma_start(binsB_t[:], binsB_d[:])
            ones_t = small.tile([128, 1], dt.float16)
            nc.sync.dma_start(ones_t[:], ones_d[:])
            eps_t = small.tile([128, 1], dt.float32)
            nc.vector.memset(eps_t[:], 1e-8)

            kt = {}
            for bi in range(2):
                for bo in range(2):
                    kt[bi, bo] = small.tile(
                        [128, 128], dt.float16, tag=f"k{bi}{bo}", name=f"k{bi}{bo}"
                    )
                    nc.sync.dma_start(
                        kt[bi, bo][:],
                        kmat_d[bi * 128 : (bi + 1) * 128, bo * 128 : (bo + 1) * 128],
                    )

            # broadcast dvrow to all partitions via K=1 ones-matmul, and
            # build the fused one-hot oh[p, hb, r, c] (fp16, exact ints)
            # chunk-by-chunk so DVE overlaps PE/ACT
            dv_bc = big.tile([128, NPIX], dt.float16, tag="dv_bc")
            oh = scratch.tile([128, 2 * NPIX], dt.float16, tag="oh")
            boff = 0
            while boff < NPIX:
                bw = min(CHUNK, NPIX - boff)
                bc_ps = psum.tile([128, bw], dt.float32, tag="g_ps", name="bc_ps")
                nc.tensor.matmul(
                    bc_ps[:], onesrow_t[:], dvrow[:, boff : boff + bw],
                    start=True, stop=True,
                )
                nc.scalar.copy(dv_bc[:, boff : boff + bw], bc_ps[:])
                nc.vector.tensor_scalar(
                    oh[:, boff : boff + bw],
                    dv_bc[:, boff : boff + bw], binsA_t[:], None, Alu.is_equal,
                )
                nc.vector.tensor_scalar(
                    oh[:, NPIX + boff : NPIX + boff + bw],
                    dv_bc[:, boff : boff + bw], binsB_t[:], None, Alu.is_equal,
                )
                boff += bw
            oh4 = oh[:].rearrange("p (h r c) -> p h r c", h=2, r=ROWS, c=80)

            # vertical 7-sum tree over r (43 -> 37): 7 = 4 + 2 + 1
            v1 = scratch.tile([128, 2 * 42 * 80], dt.float16, tag="v1")
            v1v = v1[:].rearrange("p (h r c) -> p h r c", h=2, r=42, c=80)
            nc.vector.tensor_add(v1v, oh4[:, :, 0:42, :], oh4[:, :, 1:43, :])
            v2 = scratch.tile([128, 2 * 40 * 80], dt.float16, tag="v2")
            v2v = v2[:].rearrange("p (h r c) -> p h r c", h=2, r=40, c=80)
            nc.vector.tensor_add(v2v, v1v[:, :, 0:40, :], v1v[:, :, 2:42, :])
            u2 = scratch.tile([128, 2 * PR * 80], dt.float16, tag="u2")
            u2v = u2[:].rearrange("p (h r c) -> p h r c", h=2, r=PR, c=80)
            nc.vector.tensor_add(u2v, v2v[:, :, 0:PR, :], v1v[:, :, 4 : 4 + PR, :])
            v7 = scratch.tile([128, 2 * PR * 80], dt.float16, tag="v7")
            v7v = v7[:].rearrange("p (h r c) -> p h r c", h=2, r=PR, c=80)
            nc.vector.tensor_add(v7v, u2v, oh4[:, :, 6:43, :])

            # horizontal 7-sum tree over c (80 -> 74)
            t1h = scratch.tile([128, 2 * PR * 79], dt.float16, tag="t1h")
            t1v = t1h[:].rearrange("p (h r c) -> p h r c", h=2, r=PR, c=79)
            nc.vector.tensor_add(t1v, v7v[:, :, :, 0:79], v7v[:, :, :, 1:80])
            t2h = scratch.tile([128, 2 * PR * 77], dt.float16, tag="t2h")
            t2v = t2h[:].rearrange("p (h r c) -> p h r c", h=2, r=PR, c=77)
            nc.vector.tensor_add(t2v, t1v[:, :, :, 0:77], t1v[:, :, :, 2:79])
            uh = scratch.tile([128, 2 * PR * HP], dt.float16, tag="uh")
            uhv = uh[:].rearrange("p (h r c) -> p h r c", h=2, r=PR, c=HP)
            nc.vector.tensor_add(uhv, t2v[:, :, :, 0:HP], t1v[:, :, :, 4 : 4 + HP])
            h_f = big.tile([128, 2 * NP_], dt.float16, tag="h_f")
            hfv = h_f[:].rearrange("p (h r c) -> p h r c", h=2, r=PR, c=HP)
            nc.vector.tensor_add(hfv, uhv, v7v[:, :, :, 6:80])
            # per-half flat views: h0 = h_f[:, 0:NP_], h1 = h_f[:, NP_:2*NP_]

            # ---------- stage C: chunk-pipelined G -> Ln -> prod -> reduce ---
            ent_row = small.tile([1, NP_], dt.float32)
            for off, cw in chunks:
                h0c = h_f[:, off : off + cw]
                h1c = h_f[:, NP_ + off : NP_ + off + cw]
                g0 = psum.tile([128, cw], dt.float32, tag="g_ps", name="g0")
                nc.tensor.matmul(g0[:], kt[0, 0][:], h0c, start=True, stop=False)
                nc.tensor.matmul(g0[:], kt[1, 0][:], h1c, start=False, stop=True)
                g1 = psum.tile([128, cw], dt.float32, tag="g_ps", name="g1")
                nc.tensor.matmul(g1[:], kt[0, 1][:], h0c, start=True, stop=False)
                nc.tensor.matmul(g1[:], kt[1, 1][:], h1c, start=False, stop=True)
                lp0 = scratch.tile([128, cw], dt.float16, tag="lp0", name="lp0", bufs=3)
                nc.scalar.activation(
                    lp0[:], g0[:], Act.Ln, bias=eps_t[:], scale=LN_SCALE
                )
                lp1 = scratch.tile([128, cw], dt.float16, tag="lp1", name="lp1", bufs=3)
                nc.scalar.activation(
                    lp1[:], g1[:], Act.Ln, bias=eps_t[:], scale=LN_SCALE
                )
                m0 = scratch.tile([128, cw], dt.float16, tag="m0", name="m0", bufs=3)
                nc.vector.tensor_mul(m0[:], h0c, lp0[:])
                m1 = scratch.tile([128, cw], dt.float16, tag="m1", name="m1", bufs=3)
                nc.vector.tensor_mul(m1[:], h1c, lp1[:])
                psc = scratch.tile([128, cw], dt.float16, tag="psc", name="psc", bufs=3)
                nc.vector.tensor_add(psc[:], m0[:], m1[:])
                e_ps = psum1.tile([1, cw], dt.float32, tag="mps")
                nc.tensor.matmul(e_ps[:], ones_t[:], psc[:], start=True, stop=True)
                nc.scalar.mul(ent_row[:, off : off + cw], e_ps[:], NEG_INV_L)

            nc.sync.dma_start(ent_d[:], ent_row[:])

    nc.compile()
    return nc


def _get_compiled():
    global _COMPILED
    if _COMPILED is None:
        _COMPILED = (_build_nc(), _host_constants())
    return _COMPILED


def _run(x, trace=False, **kw):
    """x: (2,2,1,80,80) float32. Returns (results, BassKernelResults)."""
    nc, consts = _get_compiled()
    xi = np.ascontiguousarray(np.asarray(x, f32).reshape(4, 80, 80))
    in_maps = []
    for core in range(8):
        b, half = core // 2, core % 2
        r0 = half * PR
        strip = np.zeros((47, 80), f32)
        lo, hi = r0 - 2, r0 + 45
        slo, shi = max(lo, 0), min(hi, 80)
        strip[slo - lo : shi - lo] = xi[b, slo:shi]
        m = dict(consts)
        m["xs"] = strip
        m["xm"] = np.ascontiguousarray(strip[2 : 2 + ROWS])
        in_maps.append(m)
    res = run_bass_kernel_spmd(nc, in_maps, list(range(8)), trace=trace, **kw)
    return res


def kernel(x):
    res = _run(x)
    out = np.zeros((4, 80, 80), f32)
    pad = R // 2
    for core in range(8):
        b, half = core // 2, core % 2
        r0 = half * PR
        ent = np.asarray(res.results[core]["ent"], f32).reshape(PR, HP)
        out[b, pad + r0 : pad + r0 + PR, pad : pad + HP] = ent
    return out.reshape(2, 2, 80, 80)
